# revision 5
# baseline (speedup 1.0000x reference)
"""Trainium2 Bass kernel for ConvDownsample2d — fp8 DoubleRow version.

Contract: kernel(**inputs) takes FULL inputs (x[16,512,64,64] f32, weight[512,512,3,3],
bias[512], fir[4,4]) and returns the FULL output [16,512,32,32] f32.

Strategy:
  - Data-parallel over batch: 2 images per core, no collectives.
  - The separable FIR [1,3,3,1]^2 is split: [1,1] (vertical) is folded into the conv
    weights on host (3x3 -> 4x3 taps); the device blurs with [1,2,1] vertically and
    [1,3,3,1] horizontally in fp16 on VectorE (5 ops), then casts to fp8e4 on ScalarE.
  - The reference zero-pads y at row/col -1. Horizontally a zero guard column handles
    it; vertically the fold breaks padding at the top row, fixed exactly by writing
    h8[-2] := -h8[-1] (then the folded pair h8[-2]+h8[-1] = 0).
  - Conv: fp8 DoubleRow matmuls (2 K-tiles of 128 cins per instruction, 2x rate).
    Weights fp8 with error-feedback rounding across the 12 taps (host, exact).
    Accuracy of the whole scheme measured offline: rel_err ~3.2e-3 (gate 2e-2).
  - Epilogue: ScalarE scale+bias from PSUM (fp16 out), VectorE leaky via max(0.2t,t),
    fp16 DMA out, host casts to f32.
"""

import sys

for p in ("/opt/trn_rl_repo", "/opt/pypackages"):
    if p not in sys.path:
        sys.path.insert(0, p)

import numpy as np
import ml_dtypes
from contextlib import ExitStack

from concourse import bass, bacc, mybir, tile
from concourse.bass_utils import run_bass_kernel_spmd

F16 = mybir.dt.float16
F32 = mybir.dt.float32
F8 = mybir.dt.float8e4
NPF8 = ml_dtypes.float8_e4m3

NCORES = 8
NPC = 2            # images per core
CIN = 512
COUT = 512
H = W = 64
OH = OW = 32
KS = 3
W_LRMUL = 1.0 / np.sqrt(CIN * COUT * KS * KS)
SQRT2 = np.sqrt(2.0)
YS = 16.0          # activation scale folded into x (fir gain 1/64 * YS)

MT = ML = 4        # top/left margin of padded SBUF tiles
SH = SW = 70       # fp16 tile extent
SW8 = 72           # fp8 tile innermost (div 4 for memzero)

_CACHE = {}


def _build(reps=1):
    nc = bacc.Bacc("TRN2", target_bir_lowering=False, debug=False, enable_asserts=False)

    x_d = nc.dram_tensor("x", [NPC, CIN, H, W], F16, kind="ExternalInput")
    w_d = nc.dram_tensor("w", [128, 2, 12, 2, COUT], F8, kind="ExternalInput")
    b_d = nc.dram_tensor("b", [128, 4], F32, kind="ExternalInput")
    o_d = nc.dram_tensor("out", [NPC, COUT, OH, OW], F16, kind="ExternalOutput")

    AL = mybir.AluOpType
    ACT = mybir.ActivationFunctionType
    DR = mybir.MatmulPerfMode.DoubleRow
    SC = float(W_LRMUL * SQRT2 / YS)

    with tile.TileContext(nc) as tc, ExitStack() as ctx:
        cpool = ctx.enter_context(tc.tile_pool(name="const", bufs=1))
        bpool = ctx.enter_context(tc.tile_pool(name="blur", bufs=1))
        opool = ctx.enter_context(tc.tile_pool(name="outp", bufs=4))
        ppool = ctx.enter_context(
            tc.tile_pool(name="psum", bufs=1, space=bass.MemorySpace.PSUM)
        )

        # --- constants ---
        w_sb = cpool.tile([128, 2, 12, 2, COUT], F8, name="w_sb")
        nc.sync.dma_start(out=w_sb[:], in_=w_d[:])
        b_sb = cpool.tile([128, 4], F32, name="b_sb")
        nc.sync.dma_start(out=b_sb[:], in_=b_d[:])

        # --- static double-buffered tiles ---
        def pair(tag, shape, dt):
            return [
                bpool.tile([128, *shape], dt, tag=f"{tag}{i}", name=f"{tag}{i}")
                for i in range(2)
            ]

        xt = pair("xt", [SH, SW], F16)
        tat = pair("ta", [SH, SW], F16)
        vt = pair("vt", [SH, SW], F16)
        t1t = pair("t1", [SH, SW], F16)
        t2t = pair("t2", [SH, SW], F16)
        ht = pair("ht", [SH, SW], F16)
        h8t = pair("h8", [4, SH, SW8], F8)

        # zero guards once; later writes stay in the interior
        for tl in (*xt, *vt, *h8t):
            nc.scalar.memzero(tl[:])

        # row/col ranges (tile coords): x rows MT..MT+63; h rows MT-2..MT+63
        A = slice(None)
        RV = slice(MT - 2, MT + 64)   # 66 rows of v/h space
        CI = slice(ML, ML + 64)

        for n in [i % NPC for i in range(reps * NPC)]:
            s_ = (n % 2)
            x_, ta, v, t1, t2, h16 = xt[s_], tat[s_], vt[s_], t1t[s_], t2t[s_], ht[s_]
            h8 = h8t[s_]
            for kc in range(4):
                cs = x_d[n, kc * 128:(kc + 1) * 128]
                nc.sync.dma_start(out=x_[:, MT:MT + 64, ML:ML + 64], in_=cs)

                # vertical residual [1,2,1]: v[r] = x[r-1] + 2 x[r] + x[r+1]
                nc.vector.tensor_tensor(
                    ta[A, RV, CI], x_[:, MT - 3:MT + 63, CI], x_[:, MT - 1:MT + 65, CI],
                    AL.add)
                nc.vector.scalar_tensor_tensor(
                    v[A, RV, CI], x_[:, MT - 2:MT + 64, CI], 2.0, ta[A, RV, CI],
                    AL.mult, AL.add)

                # horizontal [1,3,3,1]: h[c] = v[c-2] + 3 v[c-1] + 3 v[c] + v[c+1]
                nc.vector.tensor_tensor(
                    t1[A, RV, CI], v[A, RV, ML - 2:ML + 62], v[A, RV, ML + 1:ML + 65],
                    AL.add)
                nc.vector.tensor_tensor(
                    t2[A, RV, CI], v[A, RV, CI], v[A, RV, ML - 1:ML + 63], AL.add)
                nc.vector.scalar_tensor_tensor(
                    h16[A, RV, CI], t2[A, RV, CI], 3.0, t1[A, RV, CI], AL.mult, AL.add)

                # cast to fp8 plane kc; then top boundary: h8[-2] := -h8[-1]
                nc.scalar.copy(h8[:, kc, MT - 2:MT + 64, CI], h16[A, RV, CI])
                nc.scalar.activation(
                    h8[:, kc, MT - 2, CI], h8[:, kc, MT - 1, CI],
                    ACT.Identity, scale=-1.0)

            # conv: 8 psum banks [128,16,32]; each holds two 8-row accumulation
            # groups of 24 DoubleRow matmuls (out free must be 256)
            for mc in range(4):
                for uh in range(2):
                    ps = ppool.tile([128, 16, OW], F32, tag=f"ps{mc}{uh}",
                                    name=f"ps{mc}{uh}")
                    for s in range(2):
                        sblk = 2 * uh + s
                        idx = 0
                        for pp in range(4):
                            for q in range(3):
                                t = pp * 3 + q
                                r0 = MT + 16 * sblk + pp - 2
                                c0 = ML + q - 1
                                for pr in range(2):
                                    nc.tensor.matmul(
                                        ps[:, 8 * s:8 * s + 8, :],
                                        w_sb[:, pr, t, :, mc * 128:(mc + 1) * 128],
                                        h8[:, 2 * pr:2 * pr + 2, r0:r0 + 16:2,
                                           c0:c0 + 64:2],
                                        start=(idx == 0), stop=(idx == 23),
                                        perf_mode=DR)
                                    idx += 1
                    tb = opool.tile([128, 16, OW], F16, tag="tb", name="tb")
                    nc.scalar.activation(
                        tb[:], ps[:], ACT.Identity,
                        bias=b_sb[:, mc:mc + 1], scale=SC)
                    ob = opool.tile([128, 16, OW], F16, tag="ob", name="ob")
                    nc.vector.scalar_tensor_tensor(
                        ob[:], tb[:], 0.2, tb[:], AL.mult, AL.max)
                    nc.sync.dma_start(
                        out=o_d[n, mc * 128:(mc + 1) * 128,
                                uh * 16:(uh + 1) * 16, :],
                        in_=ob[:])

    nc.compile()
    return nc


def get_nc(reps=1):
    key = f"nc{reps}"
    if key not in _CACHE:
        _CACHE[key] = _build(reps)
    return _CACHE[key]


def _fp8(a):
    return a.astype(NPF8).astype(np.float32)


def _ef_round(Wb, C):
    """Error-feedback fp8 rounding: per block choose nearest/next fp8 per tap to
    minimize r^T C r (greedy coordinate descent). Wb [B,T] f32."""
    w8n = _fp8(Wb)
    d = Wb - w8n
    ulp_dir = np.where(d > 0, 1, -1).astype(np.float32)
    w8o = np.where(
        d == 0, w8n,
        np.asarray(np.nextafter(w8n.astype(NPF8), (w8n + ulp_dir * 1e6).astype(NPF8)),
                   np.float32))
    D0 = w8n - Wb
    D1 = w8o - Wb
    cur = D0.copy()
    sel = np.zeros_like(D0, bool)
    g = cur @ C.T
    Cd = np.diag(C)
    for _ in range(24):
        nf = 0
        for i in range(Wb.shape[1]):
            delta = np.where(sel[:, i], D0[:, i] - D1[:, i], D1[:, i] - D0[:, i])
            dc = 2 * delta * g[:, i] + Cd[i] * delta * delta
            flip = dc < 0
            nf += int(flip.sum())
            if flip.any():
                g += (delta * flip)[:, None] * C[i][None, :]
                cur[:, i] += delta * flip
                sel[flip, i] = ~sel[flip, i]
        if nf == 0:
            break
    return np.where(sel, w8o, w8n)


def _tap_cov():
    """C[t,t'] = autocorr of the device blur kernel outer([1,2,1],[1,3,3,1]) at the
    displacement between taps t=(pp,q), t'=(pp',q')."""
    g = np.outer([1.0, 2.0, 1.0], [1.0, 3.0, 3.0, 1.0])
    g = g / g.sum()
    taps = [(pp, q) for pp in range(4) for q in range(3)]
    C = np.zeros((12, 12), np.float32)
    for a, (p1, q1) in enumerate(taps):
        for b, (p2, q2) in enumerate(taps):
            dr, dc = p1 - p2, q1 - q2
            s = 0.0
            for r in range(3):
                for c in range(4):
                    r2, c2 = r + dr, c + dc
                    if 0 <= r2 < 3 and 0 <= c2 < 4:
                        s += g[r, c] * g[r2, c2]
            C[a, b] = s
    return C


def prep_inputs(x, weight, bias, fir):
    """Host-side shard + fold constants. Returns per-core input maps."""
    x = np.asarray(x, dtype=np.float32)
    weight = np.asarray(weight, dtype=np.float32)
    bias = np.asarray(bias, dtype=np.float32)

    # x scaled so device h = YS * (normalized blur): fir00 = 1/64 folded with YS
    x_dev = (x * np.float32(YS / 64.0)).astype(np.float16)

    # fold [1,1] vertically: Wp[cout,cin,pp,q] = sum_p w[p,q], pp-p in {0,1}
    Wp = np.zeros((COUT, CIN, 4, 3), np.float32)
    for pp in range(4):
        for p in range(3):
            if 0 <= pp - p <= 1:
                Wp[:, :, pp] += weight[:, :, p]

    W8 = _ef_round(Wp.reshape(COUT * CIN, 12), _tap_cov()).reshape(COUT, CIN, 4, 3)

    # device layout [part, pr, tap, e, cout]; cin = 256*pr + 128*e + part
    w8t = np.ascontiguousarray(
        W8.reshape(COUT, 2, 2, 128, 12).transpose(3, 1, 4, 2, 0)).astype(NPF8)

    b_host = np.ascontiguousarray(
        (bias * np.float32(SQRT2)).astype(np.float32).reshape(4, 128).T)

    in_maps = []
    for c in range(NCORES):
        in_maps.append(
            {
                "x": np.ascontiguousarray(x_dev[c * NPC:(c + 1) * NPC]),
                "w": w8t,
                "b": b_host,
            }
        )
    return in_maps


def run(in_maps, trace=False, **kw):
    nc = get_nc()
    return run_bass_kernel_spmd(nc, in_maps, list(range(NCORES)), trace=trace, **kw)


def kernel(x, weight, bias, fir):
    res = run(prep_inputs(x, weight, bias, fir)).results
    out = np.concatenate([r["out"] for r in res], axis=0)
    return out.astype(np.float32)


# revision 6
# speedup vs baseline: 1.0515x; 1.0515x over previous
"""Trainium2 Bass kernel for ConvDownsample2d — fp8 DoubleRow version.

Contract: kernel(**inputs) takes FULL inputs (x[16,512,64,64] f32, weight[512,512,3,3],
bias[512], fir[4,4]) and returns the FULL output [16,512,32,32] f32.

Strategy:
  - Data-parallel over batch: 2 images per core, no collectives.
  - The separable FIR [1,3,3,1]^2 is split: [1,1] (vertical) is folded into the conv
    weights on host (3x3 -> 4x3 taps); the device blurs with [1,2,1] vertically and
    [1,3,3,1] horizontally in fp16 on VectorE (5 ops), then casts to fp8e4 on ScalarE.
  - The reference zero-pads y at row/col -1. Horizontally a zero guard column handles
    it; vertically the fold breaks padding at the top row, fixed exactly by writing
    h8[-2] := -h8[-1] (then the folded pair h8[-2]+h8[-1] = 0).
  - Conv: fp8 DoubleRow matmuls (2 K-tiles of 128 cins per instruction, 2x rate).
    Weights fp8 with error-feedback rounding across the 12 taps (host, exact).
    Accuracy of the whole scheme measured offline: rel_err ~3.2e-3 (gate 2e-2).
  - Epilogue: ScalarE scale+bias from PSUM (fp16 out), VectorE leaky via max(0.2t,t),
    fp16 DMA out, host casts to f32.
"""

import sys

for p in ("/opt/trn_rl_repo", "/opt/pypackages"):
    if p not in sys.path:
        sys.path.insert(0, p)

import numpy as np
import ml_dtypes
from contextlib import ExitStack

from concourse import bass, bacc, mybir, tile
from concourse.bass_utils import run_bass_kernel_spmd

F16 = mybir.dt.float16
F32 = mybir.dt.float32
F8 = mybir.dt.float8e4
NPF8 = ml_dtypes.float8_e4m3

NCORES = 8
NPC = 2            # images per core
CIN = 512
COUT = 512
H = W = 64
OH = OW = 32
KS = 3
W_LRMUL = 1.0 / np.sqrt(CIN * COUT * KS * KS)
SQRT2 = np.sqrt(2.0)
YS = 16.0          # activation scale folded into x (fir gain 1/64 * YS)

MT = ML = 4        # top/left margin of padded SBUF tiles
SH = SW = 70       # fp16 tile extent
SW8 = 72           # fp8 tile innermost (div 4 for memzero)

_CACHE = {}


def _build(reps=1):
    nc = bacc.Bacc("TRN2", target_bir_lowering=False, debug=False, enable_asserts=False)

    x_d = nc.dram_tensor("x", [NPC, CIN, H, W], F16, kind="ExternalInput")
    w_d = nc.dram_tensor("w", [128, 2, 12, 2, COUT], F8, kind="ExternalInput")
    b_d = nc.dram_tensor("b", [128, 4], F32, kind="ExternalInput")
    o_d = nc.dram_tensor("out", [NPC, COUT, OH, OW], F16, kind="ExternalOutput")

    AL = mybir.AluOpType
    ACT = mybir.ActivationFunctionType
    DR = mybir.MatmulPerfMode.DoubleRow
    SC = float(W_LRMUL * SQRT2 / YS)

    with tile.TileContext(nc) as tc, ExitStack() as ctx:
        cpool = ctx.enter_context(tc.tile_pool(name="const", bufs=1))
        bpool = ctx.enter_context(tc.tile_pool(name="blur", bufs=1))
        opool = ctx.enter_context(tc.tile_pool(name="outp", bufs=4))
        ppool = ctx.enter_context(
            tc.tile_pool(name="psum", bufs=1, space=bass.MemorySpace.PSUM)
        )

        # --- constants ---
        w_sb = cpool.tile([128, 2, 12, 2, COUT], F8, name="w_sb")
        nc.sync.dma_start(out=w_sb[:], in_=w_d[:])
        b_sb = cpool.tile([128, 4], F32, name="b_sb")
        nc.sync.dma_start(out=b_sb[:], in_=b_d[:])

        # --- static double-buffered tiles ---
        def pair(tag, shape, dt):
            return [
                bpool.tile([128, *shape], dt, tag=f"{tag}{i}", name=f"{tag}{i}")
                for i in range(2)
            ]

        xt = pair("xt", [SH, SW], F16)
        tat = pair("ta", [SH, SW], F16)
        vt = pair("vt", [SH, SW], F16)
        t1t = pair("t1", [SH, SW], F16)
        t2t = pair("t2", [SH, SW], F16)
        ht = pair("ht", [SH, SW], F16)
        h8t = pair("h8", [4, SH, SW8], F8)

        # zero guards once; later writes stay in the interior
        for tl in (*xt, *vt, *h8t):
            nc.scalar.memzero(tl[:])

        # row/col ranges (tile coords): x rows MT..MT+63; h rows MT-2..MT+63
        A = slice(None)
        RV = slice(MT - 2, MT + 64)   # 66 rows of v/h space
        CI = slice(ML, ML + 64)

        for n in [i % NPC for i in range(reps * NPC)]:
            s_ = (n % 2)
            x_, ta, v, t1, t2, h16 = xt[s_], tat[s_], vt[s_], t1t[s_], t2t[s_], ht[s_]
            h8 = h8t[s_]
            for kc in range(4):
                cs = x_d[n, kc * 128:(kc + 1) * 128]
                nc.sync.dma_start(out=x_[:, MT:MT + 64, ML:ML + 64], in_=cs)

                # vertical residual [1,2,1]: v[r] = x[r-1] + 2 x[r] + x[r+1]
                nc.vector.tensor_tensor(
                    ta[A, RV, CI], x_[:, MT - 3:MT + 63, CI], x_[:, MT - 1:MT + 65, CI],
                    AL.add)
                nc.vector.scalar_tensor_tensor(
                    v[A, RV, CI], x_[:, MT - 2:MT + 64, CI], 2.0, ta[A, RV, CI],
                    AL.mult, AL.add)

                # horizontal [1,3,3,1]: h[c] = v[c-2] + 3 v[c-1] + 3 v[c] + v[c+1]
                nc.vector.tensor_tensor(
                    t1[A, RV, CI], v[A, RV, ML - 2:ML + 62], v[A, RV, ML + 1:ML + 65],
                    AL.add)
                nc.vector.tensor_tensor(
                    t2[A, RV, CI], v[A, RV, CI], v[A, RV, ML - 1:ML + 63], AL.add)
                nc.vector.scalar_tensor_tensor(
                    h16[A, RV, CI], t2[A, RV, CI], 3.0, t1[A, RV, CI], AL.mult, AL.add)

                # cast to fp8 plane kc; then top boundary: h8[-2] := -h8[-1]
                nc.scalar.copy(h8[:, kc, MT - 2:MT + 64, CI], h16[A, RV, CI])
                nc.scalar.activation(
                    h8[:, kc, MT - 2, CI], h8[:, kc, MT - 1, CI],
                    ACT.Identity, scale=-1.0)

            # conv: 8 psum banks [128,16,32], two 8-row accumulation groups each
            # (out free must be 256). Weight-major order: each lhsT is reused by
            # 4 consecutive matmuls (uh x s) to amortize PE weight loads.
            pst = {}
            for mc in range(4):
                for uh in range(2):
                    pst[mc, uh] = ppool.tile([128, 16, OW], F32, tag=f"ps{mc}{uh}",
                                             name=f"ps{mc}{uh}")
            for idx, (pp, q, pr) in enumerate(
                    (pp, q, pr) for pp in range(4) for q in range(3)
                    for pr in range(2)):
                t = pp * 3 + q
                c0 = ML + q - 1
                for mc in range(4):
                    lhsT = w_sb[:, pr, t, :, mc * 128:(mc + 1) * 128]
                    for uh in range(2):
                        for s in range(2):
                            sblk = 2 * uh + s
                            r0 = MT + 16 * sblk + pp - 2
                            nc.tensor.matmul(
                                pst[mc, uh][:, 8 * s:8 * s + 8, :],
                                lhsT,
                                h8[:, 2 * pr:2 * pr + 2, r0:r0 + 16:2,
                                   c0:c0 + 64:2],
                                start=(idx == 0), stop=(idx == 23),
                                perf_mode=DR)
            for mc in range(4):
                for uh in range(2):
                    ps = pst[mc, uh]
                    tb = opool.tile([128, 16, OW], F16, tag="tb", name="tb")
                    nc.scalar.activation(
                        tb[:], ps[:], ACT.Identity,
                        bias=b_sb[:, mc:mc + 1], scale=SC)
                    ob = opool.tile([128, 16, OW], F16, tag="ob", name="ob")
                    nc.vector.scalar_tensor_tensor(
                        ob[:], tb[:], 0.2, tb[:], AL.mult, AL.max)
                    nc.sync.dma_start(
                        out=o_d[n, mc * 128:(mc + 1) * 128,
                                uh * 16:(uh + 1) * 16, :],
                        in_=ob[:])

    nc.compile()
    return nc


def get_nc(reps=1):
    key = f"nc{reps}"
    if key not in _CACHE:
        _CACHE[key] = _build(reps)
    return _CACHE[key]


def _fp8(a):
    return a.astype(NPF8).astype(np.float32)


def _ef_round(Wb, C):
    """Error-feedback fp8 rounding: per block choose nearest/next fp8 per tap to
    minimize r^T C r (greedy coordinate descent). Wb [B,T] f32."""
    w8n = _fp8(Wb)
    d = Wb - w8n
    ulp_dir = np.where(d > 0, 1, -1).astype(np.float32)
    w8o = np.where(
        d == 0, w8n,
        np.asarray(np.nextafter(w8n.astype(NPF8), (w8n + ulp_dir * 1e6).astype(NPF8)),
                   np.float32))
    D0 = w8n - Wb
    D1 = w8o - Wb
    cur = D0.copy()
    sel = np.zeros_like(D0, bool)
    g = cur @ C.T
    Cd = np.diag(C)
    for _ in range(24):
        nf = 0
        for i in range(Wb.shape[1]):
            delta = np.where(sel[:, i], D0[:, i] - D1[:, i], D1[:, i] - D0[:, i])
            dc = 2 * delta * g[:, i] + Cd[i] * delta * delta
            flip = dc < 0
            nf += int(flip.sum())
            if flip.any():
                g += (delta * flip)[:, None] * C[i][None, :]
                cur[:, i] += delta * flip
                sel[flip, i] = ~sel[flip, i]
        if nf == 0:
            break
    return np.where(sel, w8o, w8n)


def _tap_cov():
    """C[t,t'] = autocorr of the device blur kernel outer([1,2,1],[1,3,3,1]) at the
    displacement between taps t=(pp,q), t'=(pp',q')."""
    g = np.outer([1.0, 2.0, 1.0], [1.0, 3.0, 3.0, 1.0])
    g = g / g.sum()
    taps = [(pp, q) for pp in range(4) for q in range(3)]
    C = np.zeros((12, 12), np.float32)
    for a, (p1, q1) in enumerate(taps):
        for b, (p2, q2) in enumerate(taps):
            dr, dc = p1 - p2, q1 - q2
            s = 0.0
            for r in range(3):
                for c in range(4):
                    r2, c2 = r + dr, c + dc
                    if 0 <= r2 < 3 and 0 <= c2 < 4:
                        s += g[r, c] * g[r2, c2]
            C[a, b] = s
    return C


def prep_inputs(x, weight, bias, fir):
    """Host-side shard + fold constants. Returns per-core input maps."""
    x = np.asarray(x, dtype=np.float32)
    weight = np.asarray(weight, dtype=np.float32)
    bias = np.asarray(bias, dtype=np.float32)

    # x scaled so device h = YS * (normalized blur): fir00 = 1/64 folded with YS
    x_dev = (x * np.float32(YS / 64.0)).astype(np.float16)

    # fold [1,1] vertically: Wp[cout,cin,pp,q] = sum_p w[p,q], pp-p in {0,1}
    Wp = np.zeros((COUT, CIN, 4, 3), np.float32)
    for pp in range(4):
        for p in range(3):
            if 0 <= pp - p <= 1:
                Wp[:, :, pp] += weight[:, :, p]

    W8 = _ef_round(Wp.reshape(COUT * CIN, 12), _tap_cov()).reshape(COUT, CIN, 4, 3)

    # device layout [part, pr, tap, e, cout]; cin = 256*pr + 128*e + part
    w8t = np.ascontiguousarray(
        W8.reshape(COUT, 2, 2, 128, 12).transpose(3, 1, 4, 2, 0)).astype(NPF8)

    b_host = np.ascontiguousarray(
        (bias * np.float32(SQRT2)).astype(np.float32).reshape(4, 128).T)

    in_maps = []
    for c in range(NCORES):
        in_maps.append(
            {
                "x": np.ascontiguousarray(x_dev[c * NPC:(c + 1) * NPC]),
                "w": w8t,
                "b": b_host,
            }
        )
    return in_maps


def run(in_maps, trace=False, **kw):
    nc = get_nc()
    return run_bass_kernel_spmd(nc, in_maps, list(range(NCORES)), trace=trace, **kw)


def kernel(x, weight, bias, fir):
    res = run(prep_inputs(x, weight, bias, fir)).results
    out = np.concatenate([r["out"] for r in res], axis=0)
    return out.astype(np.float32)


# revision 7
# speedup vs baseline: 1.0708x; 1.0183x over previous
"""Trainium2 Bass kernel for ConvDownsample2d — fp8 DoubleRow version.

Contract: kernel(**inputs) takes FULL inputs (x[16,512,64,64] f32, weight[512,512,3,3],
bias[512], fir[4,4]) and returns the FULL output [16,512,32,32] f32.

Strategy:
  - Data-parallel over batch: 2 images per core, no collectives.
  - The separable FIR [1,3,3,1]^2 is split: [1,1] (vertical) is folded into the conv
    weights on host (3x3 -> 4x3 taps); the device blurs with [1,2,1] vertically and
    [1,3,3,1] horizontally in fp16 on VectorE (5 ops), then casts to fp8e4 on ScalarE.
  - The reference zero-pads y at row/col -1. Horizontally a zero guard column handles
    it; vertically the fold breaks padding at the top row, fixed exactly by writing
    h8[-2] := -h8[-1] (then the folded pair h8[-2]+h8[-1] = 0).
  - Conv: fp8 DoubleRow matmuls (2 K-tiles of 128 cins per instruction, 2x rate).
    Weights fp8 with error-feedback rounding across the 12 taps (host, exact).
    Accuracy of the whole scheme measured offline: rel_err ~3.2e-3 (gate 2e-2).
  - Epilogue: ScalarE scale+bias from PSUM (fp16 out), VectorE leaky via max(0.2t,t),
    fp16 DMA out, host casts to f32.
"""

import sys

for p in ("/opt/trn_rl_repo", "/opt/pypackages"):
    if p not in sys.path:
        sys.path.insert(0, p)

import numpy as np
import ml_dtypes
from contextlib import ExitStack

from concourse import bass, bacc, mybir, tile
from concourse.bass_utils import run_bass_kernel_spmd

F16 = mybir.dt.float16
F32 = mybir.dt.float32
F8 = mybir.dt.float8e4
NPF8 = ml_dtypes.float8_e4m3

NCORES = 8
NPC = 2            # images per core
CIN = 512
COUT = 512
H = W = 64
OH = OW = 32
KS = 3
W_LRMUL = 1.0 / np.sqrt(CIN * COUT * KS * KS)
SQRT2 = np.sqrt(2.0)
YS = 16.0          # activation scale folded into x (fir gain 1/64 * YS)

MT = ML = 4        # top/left margin of padded SBUF tiles
SH = SW = 70       # fp16 tile extent
SW8 = 72           # fp8 tile innermost (div 4 for memzero)

_CACHE = {}


def _build(reps=1):
    nc = bacc.Bacc("TRN2", target_bir_lowering=False, debug=False, enable_asserts=False)

    x_d = nc.dram_tensor("x", [NPC, CIN, H, W], F16, kind="ExternalInput")
    w_d = nc.dram_tensor("w", [128, 2, 12, 2, COUT], F8, kind="ExternalInput")
    b_d = nc.dram_tensor("b", [128, 4], F32, kind="ExternalInput")
    o_d = nc.dram_tensor("out", [NPC, COUT, OH, OW], F16, kind="ExternalOutput")

    AL = mybir.AluOpType
    ACT = mybir.ActivationFunctionType
    DR = mybir.MatmulPerfMode.DoubleRow
    SC = float(W_LRMUL * SQRT2 / YS)

    with tile.TileContext(nc) as tc, ExitStack() as ctx:
        cpool = ctx.enter_context(tc.tile_pool(name="const", bufs=1))
        bpool = ctx.enter_context(tc.tile_pool(name="blur", bufs=1))
        opool = ctx.enter_context(tc.tile_pool(name="outp", bufs=4))
        ppool = ctx.enter_context(
            tc.tile_pool(name="psum", bufs=1, space=bass.MemorySpace.PSUM)
        )

        # --- constants ---
        w_sb = cpool.tile([128, 2, 12, 2, COUT], F8, name="w_sb")
        nc.sync.dma_start(out=w_sb[:], in_=w_d[:])
        b_sb = cpool.tile([128, 4], F32, name="b_sb")
        nc.sync.dma_start(out=b_sb[:], in_=b_d[:])

        # --- static double-buffered tiles ---
        def pair(tag, shape, dt):
            return [
                bpool.tile([128, *shape], dt, tag=f"{tag}{i}", name=f"{tag}{i}")
                for i in range(2)
            ]

        xt = pair("xt", [SH, SW], F16)
        tat = pair("ta", [SH, SW], F16)
        vt = pair("vt", [SH, SW], F16)
        t1t = pair("t1", [SH, SW], F16)
        t2t = pair("t2", [SH, SW], F16)
        ht = pair("ht", [SH, SW], F16)
        h8t = pair("h8", [4, 2, SH, 36], F8)

        # zero guards once; later writes stay in the interior
        for tl in (*xt, *vt, *h8t):
            nc.scalar.memzero(tl[:])

        # row/col ranges (tile coords): x rows MT..MT+63; h rows MT-2..MT+63
        A = slice(None)
        RV = slice(MT - 2, MT + 64)   # 66 rows of v/h space
        CI = slice(ML, ML + 64)

        for n in [i % NPC for i in range(reps * NPC)]:
            s_ = (n % 2)
            x_, ta, v, t1, t2, h16 = xt[s_], tat[s_], vt[s_], t1t[s_], t2t[s_], ht[s_]
            h8 = h8t[s_]
            for kc in range(4):
                cs = x_d[n, kc * 128:(kc + 1) * 128]
                nc.sync.dma_start(out=x_[:, MT:MT + 64, ML:ML + 64], in_=cs)

                # vertical residual [1,2,1]: v[r] = x[r-1] + 2 x[r] + x[r+1]
                nc.vector.tensor_tensor(
                    ta[A, RV, CI], x_[:, MT - 3:MT + 63, CI], x_[:, MT - 1:MT + 65, CI],
                    AL.add)
                nc.vector.scalar_tensor_tensor(
                    v[A, RV, CI], x_[:, MT - 2:MT + 64, CI], 2.0, ta[A, RV, CI],
                    AL.mult, AL.add)

                # horizontal [1,3,3,1]: h[c] = v[c-2] + 3 v[c-1] + 3 v[c] + v[c+1]
                nc.vector.tensor_tensor(
                    t1[A, RV, CI], v[A, RV, ML - 2:ML + 62], v[A, RV, ML + 1:ML + 65],
                    AL.add)
                nc.vector.tensor_tensor(
                    t2[A, RV, CI], v[A, RV, CI], v[A, RV, ML - 1:ML + 63], AL.add)
                nc.vector.scalar_tensor_tensor(
                    h16[A, RV, CI], t2[A, RV, CI], 3.0, t1[A, RV, CI], AL.mult, AL.add)

                # cast to fp8, column-polyphase: par0 j=(c+1)/2 (odd c), par1 j=c/2
                nc.scalar.copy(h8[:, kc, 0, MT - 2:MT + 64, 1:33],
                               h16[A, RV, ML + 1:ML + 64:2])
                nc.scalar.copy(h8[:, kc, 1, MT - 2:MT + 64, 0:32],
                               h16[A, RV, ML:ML + 63:2])
                # top boundary: h8[-2] := -h8[-1] (both parity planes)
                nc.scalar.activation(
                    h8[:, kc, :, MT - 2, :], h8[:, kc, :, MT - 1, :],
                    ACT.Identity, scale=-1.0)

            # conv: 8 psum banks [128,16,32], two sequential 8-row accumulation
            # groups each (out free must be 256). rhs innermost dim contiguous
            # (column-polyphase) so PE matmuls pipeline back-to-back.
            PQJ = {0: (0, 0), 1: (1, 0), 2: (0, 1)}  # q -> (par, j0)
            for mc in range(4):
                for uh in range(2):
                    ps = ppool.tile([128, 16, OW], F32, tag=f"ps{mc}{uh}",
                                    name=f"ps{mc}{uh}")
                    for s in range(2):
                        sblk = 2 * uh + s
                        idx = 0
                        for pp in range(4):
                            for q in range(3):
                                t = pp * 3 + q
                                par, j0 = PQJ[q]
                                r0 = MT + 16 * sblk + pp - 2
                                for pr in range(2):
                                    nc.tensor.matmul(
                                        ps[:, 8 * s:8 * s + 8, :],
                                        w_sb[:, pr, t, :, mc * 128:(mc + 1) * 128],
                                        h8[:, 2 * pr:2 * pr + 2, par, r0:r0 + 16:2,
                                           j0:j0 + 32],
                                        start=(idx == 0), stop=(idx == 23),
                                        perf_mode=DR)
                                    idx += 1
                    tb = opool.tile([128, 16, OW], F16, tag="tb", name="tb")
                    nc.scalar.activation(
                        tb[:], ps[:], ACT.Identity,
                        bias=b_sb[:, mc:mc + 1], scale=SC)
                    ob = opool.tile([128, 16, OW], F16, tag="ob", name="ob")
                    nc.vector.scalar_tensor_tensor(
                        ob[:], tb[:], 0.2, tb[:], AL.mult, AL.max)
                    nc.sync.dma_start(
                        out=o_d[n, mc * 128:(mc + 1) * 128,
                                uh * 16:(uh + 1) * 16, :],
                        in_=ob[:])

    nc.compile()
    return nc


def get_nc(reps=1):
    key = f"nc{reps}"
    if key not in _CACHE:
        _CACHE[key] = _build(reps)
    return _CACHE[key]


def _fp8(a):
    return a.astype(NPF8).astype(np.float32)


def _ef_round(Wb, C):
    """Error-feedback fp8 rounding: per block choose nearest/next fp8 per tap to
    minimize r^T C r (greedy coordinate descent). Wb [B,T] f32."""
    w8n = _fp8(Wb)
    d = Wb - w8n
    ulp_dir = np.where(d > 0, 1, -1).astype(np.float32)
    w8o = np.where(
        d == 0, w8n,
        np.asarray(np.nextafter(w8n.astype(NPF8), (w8n + ulp_dir * 1e6).astype(NPF8)),
                   np.float32))
    D0 = w8n - Wb
    D1 = w8o - Wb
    cur = D0.copy()
    sel = np.zeros_like(D0, bool)
    g = cur @ C.T
    Cd = np.diag(C)
    for _ in range(24):
        nf = 0
        for i in range(Wb.shape[1]):
            delta = np.where(sel[:, i], D0[:, i] - D1[:, i], D1[:, i] - D0[:, i])
            dc = 2 * delta * g[:, i] + Cd[i] * delta * delta
            flip = dc < 0
            nf += int(flip.sum())
            if flip.any():
                g += (delta * flip)[:, None] * C[i][None, :]
                cur[:, i] += delta * flip
                sel[flip, i] = ~sel[flip, i]
        if nf == 0:
            break
    return np.where(sel, w8o, w8n)


def _tap_cov():
    """C[t,t'] = autocorr of the device blur kernel outer([1,2,1],[1,3,3,1]) at the
    displacement between taps t=(pp,q), t'=(pp',q')."""
    g = np.outer([1.0, 2.0, 1.0], [1.0, 3.0, 3.0, 1.0])
    g = g / g.sum()
    taps = [(pp, q) for pp in range(4) for q in range(3)]
    C = np.zeros((12, 12), np.float32)
    for a, (p1, q1) in enumerate(taps):
        for b, (p2, q2) in enumerate(taps):
            dr, dc = p1 - p2, q1 - q2
            s = 0.0
            for r in range(3):
                for c in range(4):
                    r2, c2 = r + dr, c + dc
                    if 0 <= r2 < 3 and 0 <= c2 < 4:
                        s += g[r, c] * g[r2, c2]
            C[a, b] = s
    return C


def prep_inputs(x, weight, bias, fir):
    """Host-side shard + fold constants. Returns per-core input maps."""
    x = np.asarray(x, dtype=np.float32)
    weight = np.asarray(weight, dtype=np.float32)
    bias = np.asarray(bias, dtype=np.float32)

    # x scaled so device h = YS * (normalized blur): fir00 = 1/64 folded with YS
    x_dev = (x * np.float32(YS / 64.0)).astype(np.float16)

    # fold [1,1] vertically: Wp[cout,cin,pp,q] = sum_p w[p,q], pp-p in {0,1}
    Wp = np.zeros((COUT, CIN, 4, 3), np.float32)
    for pp in range(4):
        for p in range(3):
            if 0 <= pp - p <= 1:
                Wp[:, :, pp] += weight[:, :, p]

    W8 = _ef_round(Wp.reshape(COUT * CIN, 12), _tap_cov()).reshape(COUT, CIN, 4, 3)

    # device layout [part, pr, tap, e, cout]; cin = 256*pr + 128*e + part
    w8t = np.ascontiguousarray(
        W8.reshape(COUT, 2, 2, 128, 12).transpose(3, 1, 4, 2, 0)).astype(NPF8)

    b_host = np.ascontiguousarray(
        (bias * np.float32(SQRT2)).astype(np.float32).reshape(4, 128).T)

    in_maps = []
    for c in range(NCORES):
        in_maps.append(
            {
                "x": np.ascontiguousarray(x_dev[c * NPC:(c + 1) * NPC]),
                "w": w8t,
                "b": b_host,
            }
        )
    return in_maps


def run(in_maps, trace=False, **kw):
    nc = get_nc()
    return run_bass_kernel_spmd(nc, in_maps, list(range(NCORES)), trace=trace, **kw)


def kernel(x, weight, bias, fir):
    res = run(prep_inputs(x, weight, bias, fir)).results
    out = np.concatenate([r["out"] for r in res], axis=0)
    return out.astype(np.float32)


# revision 8
# speedup vs baseline: 1.3555x; 1.2659x over previous
"""Trainium2 Bass kernel for ConvDownsample2d (FIR blur + 3x3/s2 conv + bias + leaky_relu*sqrt2).

Contract: kernel(**inputs) takes FULL inputs (x[16,512,64,64] f32, weight[512,512,3,3],
bias[512], fir[4,4]) and returns the FULL output [16,512,32,32] f32.

Strategy (hardcoded for this problem size):
  - Data-parallel over batch: 16 images / 8 cores = 2 images per core. No collectives.
  - Host prep: x scaled by fir[0,0] (=1/64) and cast to fp16; weights transposed to
    [cin, 3*3, cout], scaled by W_LRMUL*sqrt2, cast fp16; bias*sqrt2 as [128,4] f32.
  - Device: separable [1,3,3,1] blur on VectorE in fp16 (6 ops/chunk, all operands kept
    4B-aligned via a one-element-shifted second DMA copy of x), then the strided conv as
    accumulated 128x128x512 fp16 matmuls on TensorE (channels on partitions, 9 taps x
    4 cin-chunks into PSUM), epilogue bias+leaky_relu(0.2) on ScalarE, DMA out f32.
"""

import sys

for p in ("/opt/trn_rl_repo", "/opt/pypackages"):
    if p not in sys.path:
        sys.path.insert(0, p)

import numpy as np
from contextlib import ExitStack

import ml_dtypes
from concourse import bass, bacc, mybir, tile
from concourse.bass_utils import run_bass_kernel_spmd

F16 = mybir.dt.float16
F32 = mybir.dt.float32
F8 = mybir.dt.float8e4
NPF8 = ml_dtypes.float8_e4m3
YS = 16.0

NCORES = 8
NPC = 2            # images per core
CIN = 512
COUT = 512
H = W = 64
OH = OW = 32
KS = 3
W_LRMUL = 1.0 / np.sqrt(CIN * COUT * KS * KS)
SQRT2 = np.sqrt(2.0)

MT = ML = 4        # top/left margin of padded SBUF tiles
SH = SW = 70       # padded tile extent (4 + 64 + 2)

_CACHE = {}


def _build(reps=1):
    nc = bacc.Bacc("TRN2", target_bir_lowering=False, debug=False, enable_asserts=False)

    x_d = nc.dram_tensor("x", [NPC, CIN, H, W], F16, kind="ExternalInput")
    w_d = nc.dram_tensor("w", [CIN, 9, COUT], F8, kind="ExternalInput")
    b_d = nc.dram_tensor("b", [128, 4], F32, kind="ExternalInput")
    o_d = nc.dram_tensor("out", [NPC, COUT, OH, OW], F32, kind="ExternalOutput")
    SC = float((1.0 / np.sqrt(CIN * COUT * KS * KS)) * np.sqrt(2.0) / YS)

    with tile.TileContext(nc) as tc, ExitStack() as ctx:
        cpool = ctx.enter_context(tc.tile_pool(name="const", bufs=1))
        bpool = ctx.enter_context(tc.tile_pool(name="blur", bufs=1))
        opool = ctx.enter_context(tc.tile_pool(name="outp", bufs=4))
        ppool = ctx.enter_context(
            tc.tile_pool(name="psum", bufs=1, space=bass.MemorySpace.PSUM)
        )

        # --- constants ---
        w_sb = cpool.tile([128, 4, 9, COUT], F8, name="w_sb")
        for kc in range(4):
            nc.sync.dma_start(out=w_sb[:, kc], in_=w_d[kc * 128:(kc + 1) * 128])
        b_sb = cpool.tile([128, 4], F32, name="b_sb")
        nc.sync.dma_start(out=b_sb[:], in_=b_d[:])

        # --- static double-buffered blur tiles ---
        def pair(tag):
            return [
                bpool.tile([128, SH, SW], F16, tag=f"{tag}{i}", name=f"{tag}{i}")
                for i in range(2)
            ]

        xt, xst, t1t, t2t, zt, yt = (pair(t) for t in ("xt", "xs", "t1", "t2", "zt", "yt"))
        y8t = [
            bpool.tile([128, SH, 72], mybir.dt.float8e4, tag=f"y8{i}", name=f"y8{i}")
            for i in range(2)
        ]

        # zero guards once; every later write stays in the interior
        for tl in (*xt, *xst, *zt, *yt):
            nc.scalar.memzero(tl[:])
        for tl in y8t:
            nc.scalar.memzero(tl[:])

        AL = mybir.AluOpType

        for n in [i % NPC for i in range(reps * NPC)]:
            psum = [
                [
                    ppool.tile([128, 16, OW], F32, tag=f"ps{mc}{uh}", name=f"ps{mc}{uh}")
                    for uh in range(2)
                ]
                for mc in range(4)
            ]
            for kc in range(4):
                s = (n * 4 + kc) % 2
                x_, xs_, t1, t2, z, y = xt[s], xst[s], t1t[s], t2t[s], zt[s], yt[s]
                cs = x_d[n, kc * 128:(kc + 1) * 128]
                nc.sync.dma_start(out=x_[:, MT:MT + 64, ML:ML + 64], in_=cs)
                # xs[r,c] = x[r,c+1]: derive shifted copy on ScalarE (saves an
                # HBM re-read; keeps all VectorE blur operands 4B-aligned)
                nc.scalar.copy(
                    xs_[:, MT:MT + 64, ML - 1:ML + 63], x_[:, MT:MT + 64, ML:ML + 64]
                )

                # W-blur: z[r,c] = x[c-2] + 3 x[c-1] + 3 x[c] + x[c+1]
                I = (slice(None), slice(MT, MT + 64), slice(ML, ML + 64))
                Im2 = (slice(None), slice(MT, MT + 64), slice(ML - 2, ML + 62))
                nc.vector.tensor_tensor(t1[I], x_[Im2], xs_[I], AL.add)
                nc.vector.tensor_tensor(t2[I], x_[I], xs_[Im2], AL.add)
                nc.vector.scalar_tensor_tensor(z[I], t2[I], 3.0, t1[I], AL.mult, AL.add)

                # H-blur: y[r,c] = z[r-2] + 3 z[r-1] + 3 z[r] + z[r+1]
                def rs(dr):
                    return (slice(None), slice(MT + dr, MT + dr + 64), slice(ML, ML + 64))

                nc.vector.tensor_tensor(t1[I], z[rs(-2)], z[rs(1)], AL.add)
                nc.vector.tensor_tensor(t2[I], z[rs(-1)], z[rs(0)], AL.add)
                nc.vector.scalar_tensor_tensor(y[I], t2[I], 3.0, t1[I], AL.mult, AL.add)
                y8 = y8t[s]
                nc.scalar.copy(y8[:, MT:MT + 64, ML:ML + 64], y[I])

                # conv taps: psum[mc][uh] += w[p,q,kc,mc].T @ y[2u+p-1, 2v+q-1]
                for pq in range(9):
                    p, q = divmod(pq, 3)
                    for mc in range(4):
                        lhsT = w_sb[:, kc, pq, mc * 128:(mc + 1) * 128]
                        for uh in range(2):
                            r0 = MT - 1 + p + 32 * uh
                            c0 = ML - 1 + q
                            rhs = y8[:, r0:r0 + 32:2, c0:c0 + 64:2]
                            nc.tensor.matmul(
                                psum[mc][uh][:],
                                lhsT,
                                rhs,
                                start=(kc == 0 and pq == 0),
                                stop=(kc == 3 and pq == 8),
                                perf_mode=mybir.MatmulPerfMode.DoublePixel,
                            )

            # epilogue: out = leaky_relu_0.2(psum + bias)   (sqrt2 folded on host)
            # ScalarE adds bias (exact f32) evacuating PSUM; VectorE does
            # leaky via max(0.2*t, t) in one scalar_tensor_tensor op.
            for mc in range(4):
                for uh in range(2):
                    tb = opool.tile([128, 16, OW], F32, tag="tb", name="tb")
                    nc.scalar.activation(
                        tb[:],
                        psum[mc][uh][:],
                        mybir.ActivationFunctionType.Identity,
                        bias=b_sb[:, mc:mc + 1],
                        scale=SC,
                    )
                    ob = opool.tile([128, 16, OW], F32, tag="ob", name="ob")
                    # leaky = max(0.2v, v) in one VectorE op
                    nc.vector.scalar_tensor_tensor(
                        ob[:], tb[:], 0.2, tb[:], AL.mult, AL.max
                    )
                    nc.sync.dma_start(
                        out=o_d[n, mc * 128:(mc + 1) * 128, uh * 16:(uh + 1) * 16, :],
                        in_=ob[:],
                    )

    nc.compile()
    return nc


def get_nc(reps=1):
    key = f"nc{reps}"
    if key not in _CACHE:
        _CACHE[key] = _build(reps)
    return _CACHE[key]


def prep_inputs(x, weight, bias, fir):
    """Host-side shard + fold constants. Returns per-core input maps."""
    x = np.asarray(x, dtype=np.float32)
    weight = np.asarray(weight, dtype=np.float32)
    bias = np.asarray(bias, dtype=np.float32)
    fir = np.asarray(fir, dtype=np.float32)

    # normalized separable fir = fir[0,0] * outer([1,3,3,1],[1,3,3,1]);
    # fold fir[0,0]*YS into x, integer taps run on device; fp8 weights raw.
    scale = float(fir[0, 0]) * YS
    x_dev = (x * scale).astype(np.float16)

    w_host = np.ascontiguousarray(
        weight.transpose(1, 2, 3, 0)
        .reshape(CIN, 9, COUT)
        .astype(NPF8)
    )
    b_host = np.ascontiguousarray(
        (bias * np.float32(SQRT2)).astype(np.float32).reshape(4, 128).T
    )

    in_maps = []
    for c in range(NCORES):
        in_maps.append(
            {
                "x": np.ascontiguousarray(x_dev[c * NPC:(c + 1) * NPC]),
                "w": w_host,
                "b": b_host,
            }
        )
    return in_maps


def run(in_maps, trace=False, **kw):
    nc = get_nc()
    return run_bass_kernel_spmd(nc, in_maps, list(range(NCORES)), trace=trace, **kw)


def kernel(x, weight, bias, fir):
    res = run(prep_inputs(x, weight, bias, fir)).results
    out = np.concatenate([r["out"] for r in res], axis=0)
    return out.astype(np.float32)

